# revision 5
# baseline (speedup 1.0000x reference)
"""Trainium2 Bass kernel for causal multi-head attention (B=4, S=2048, V=1024, H=16).

Sharding over 8 NeuronCores: core c handles batch b = c//2 and head group
g = c%2 (8 heads, i.e. a 512-wide slice of the hidden dim).  Each core
computes QKV projections, causal attention and a partial output projection
against its 512-row slice of W_out.T; the host sums the two partial output
projections per batch element (the tensor-parallel all-reduce).

On-core dataflow (all matmuls in bf16 with fp32 PSUM accumulation):
  x tiles --PE transpose--> xT --matmul--> QT/KT (d-major) and V (s-major)
  scoresT[k,q] = KT_tile.T @ QT  (only causal strips, q >= 128*kt)
  attnT = exp(scores/8)  (ACT, PSUM->SBUF bf16), diagonal 128-block masked
  oT_aug[d|sum, q] = V_aug.T @ attnT  (ones column yields softmax denoms)
  oT = oT_aug * broadcast(1/sum)  (PE rank-1 broadcast + DVE multiply)
  out_partial = oT.T @ W_out.T slice  (PSUM accumulate over v chunks)
"""

import sys

if "/opt/trn_rl_repo" not in sys.path:
    sys.path.insert(0, "/opt/trn_rl_repo")

import numpy as np

S = 2048          # sequence length
HS = 64           # head size
NHL = 8           # heads per core
VSL = 512         # hidden slice per core
NKT = 16          # 128-row key tiles
NCC = 4           # 128-partition chunks of the 512-wide slice
NQT = 16          # 128-row query tiles (output projection)
V_FULL = 1024
B_FULL = 4
N_CORES = 8

_CACHE = {}


def _build():
    import concourse.mybir as mybir
    import concourse.tile as tile
    from concourse import bacc
    from concourse.bass_interp import get_hw_module
    from concourse.masks import make_identity, make_upper_triangular

    fp32 = mybir.dt.float32
    bf16 = mybir.dt.bfloat16
    Exp = mybir.ActivationFunctionType.Exp
    Ln = mybir.ActivationFunctionType.Ln

    nc = bacc.Bacc("TRN2", target_bir_lowering=False, debug=False,
                   num_devices=N_CORES)
    xs = nc.dram_tensor("xs", [S, VSL], fp32, kind="ExternalInput").ap()
    wqkvT = nc.dram_tensor("wqkvT", [HS, 3 * HS], fp32, kind="ExternalInput").ap()
    woT = nc.dram_tensor("woT", [VSL, V_FULL], fp32, kind="ExternalInput").ap()
    out = nc.dram_tensor("out", [S, V_FULL], fp32, kind="ExternalOutput").ap()

    with tile.TileContext(nc) as tc:
        with (
            tc.tile_pool(name="const", bufs=1) as pc,
            tc.tile_pool(name="stage", bufs=3) as pstg,
            tc.tile_pool(name="attn", bufs=2) as pat,
            tc.tile_pool(name="small", bufs=2) as psm,
            tc.tile_pool(name="outp", bufs=2) as pout,
            tc.tile_pool(name="ps_misc", bufs=2, space="PSUM") as pmisc,
            tc.tile_pool(name="ps_scores", bufs=2, space="PSUM") as pscr,
            tc.tile_pool(name="ps_o", bufs=2, space="PSUM") as pso,
        ):
            # ---- constants ----
            ident = pc.tile([128, 128], fp32, tag="ident")
            make_identity(nc, ident[:])
            trimask = pc.tile([128, 128], bf16, tag="trimask")
            make_upper_triangular(nc, trimask[:], val=1.0, diag=True)
            ones64 = pc.tile([1, HS], bf16, tag="ones64")
            nc.gpsimd.memset(ones64[:], 1.0)

            # qkv weights, replicated into both partition halves so lhsT and
            # rhs of the per-head matmuls share a base partition
            wq2f = pc.tile([128, 3 * HS], fp32, tag="wq2f")
            nc.sync.dma_start(wq2f[0:64, :], wqkvT[:, :])
            nc.sync.dma_start(wq2f[64:128, :], wqkvT[:, :])
            wq2 = pc.tile([128, 3 * HS], bf16, tag="wq2")
            nc.vector.tensor_copy(wq2[:], wq2f[:])

            wo_b = []
            for c in range(NCC):
                wof = pstg.tile([128, V_FULL], fp32, tag="wostage")
                nc.sync.dma_start(wof[:], woT[128 * c:128 * (c + 1), :])
                wob = pc.tile([128, V_FULL], bf16, tag=f"wo{c}")
                nc.vector.tensor_copy(wob[:], wof[:])
                wo_b.append(wob)

            # persistent bf16 activations (partition-packed head pairs)
            xT = [pc.tile([128, S], bf16, tag=f"xt{c}", name=f"xt{c}")
                  for c in range(NCC)]
            QT = [pc.tile([128, S], bf16, tag=f"qt{c}", name=f"qt{c}")
                  for c in range(NCC)]
            KT = [pc.tile([128, S], bf16, tag=f"kt{c}", name=f"kt{c}")
                  for c in range(NCC)]
            Vt = [pc.tile([128, NKT, HS + 1], bf16, tag=f"v{h}", name=f"v{h}")
                  for h in range(NHL)]
            oT = [pc.tile([128, S], bf16, tag=f"ot{c}", name=f"ot{c}")
                  for c in range(NCC)]

            # ---- phase 1: transpose x into xT (and downcast to bf16) ----
            for rt in range(NQT):
                xin = pstg.tile([128, VSL], fp32, tag="xin")
                nc.sync.dma_start(xin[:], xs[128 * rt:128 * (rt + 1), :])
                for cc in range(NCC):
                    tp = pmisc.tile([128, 128], fp32, tag="ps")
                    nc.tensor.transpose(tp[:], xin[:, 128 * cc:128 * (cc + 1)],
                                        ident[:])
                    nc.vector.tensor_copy(xT[cc][:, 128 * rt:128 * (rt + 1)], tp[:])

            # ---- phases 2+3, per head ----
            for h in range(NHL):
                c, pp = h // 2, 64 * (h % 2)
                xTh = xT[c]

                # QKV projections
                for j in range(4):
                    qp = pmisc.tile([64, 512], fp32, tag="ps")
                    nc.tensor.matmul(qp[:], wq2[pp:pp + 64, 0:HS],
                                     xTh[pp:pp + 64, 512 * j:512 * (j + 1)],
                                     start=True, stop=True)
                    nc.vector.tensor_copy(QT[c][pp:pp + 64, 512 * j:512 * (j + 1)],
                                          qp[:])
                    kp = pmisc.tile([64, 512], fp32, tag="ps")
                    nc.tensor.matmul(kp[:], wq2[pp:pp + 64, HS:2 * HS],
                                     xTh[pp:pp + 64, 512 * j:512 * (j + 1)],
                                     start=True, stop=True)
                    nc.vector.tensor_copy(KT[c][pp:pp + 64, 512 * j:512 * (j + 1)],
                                          kp[:])
                for t4 in range(NKT // 4):
                    vp = pmisc.tile([128, 4, HS], fp32, tag="ps")
                    for tt in range(4):
                        t = 4 * t4 + tt
                        nc.tensor.matmul(vp[:, tt, :],
                                         xTh[pp:pp + 64, 128 * t:128 * (t + 1)],
                                         wq2[pp:pp + 64, 2 * HS:3 * HS],
                                         start=True, stop=True)
                    nc.vector.tensor_copy(Vt[h][:, 4 * t4:4 * (t4 + 1), 0:HS],
                                          vp[:])
                nc.vector.memset(Vt[h][:, :, HS:HS + 1], 1.0)

                # scoresT strips + exp + causal mask
                strips = []
                for kt in range(NKT):
                    w = S - 128 * kt
                    strip = pat.tile([128, w], bf16, tag=f"st{kt}")
                    strips.append(strip)
                    if kt < 8:
                        pieces = [(0, 1024 - 128 * kt), (1024 - 128 * kt, 1024)]
                    else:
                        pieces = [(0, w)]
                    for off, pw in pieces:
                        sc = pscr.tile([128, 1024], fp32, tag="scr")
                        for ch in range(0, pw, 512):
                            n = min(512, pw - ch)
                            q0 = 128 * kt + off + ch
                            nc.tensor.matmul(
                                sc[:, ch:ch + n],
                                KT[c][pp:pp + 64, 128 * kt:128 * (kt + 1)],
                                QT[c][pp:pp + 64, q0:q0 + n],
                                start=True, stop=True)
                        nc.scalar.activation(strip[:, off:off + pw], sc[:, 0:pw],
                                             Exp, scale=0.125)
                    nc.vector.tensor_mul(strip[:, 0:128], strip[:, 0:128],
                                         trimask[:])

                # attn @ V_aug per 512-wide query block, then normalize
                for qb in range(4):
                    op = pso.tile([HS + 1, 512], fp32, tag="ot")
                    nk = 4 * qb + 4
                    for kt in range(nk):
                        a_off = 512 * qb - 128 * kt
                        o0 = max(0, -a_off)
                        nc.tensor.matmul(
                            op[:, o0:512],
                            Vt[h][:, kt, :],
                            strips[kt][:, max(0, a_off):512 * qb + 512 - 128 * kt],
                            start=(kt == 0), stop=(kt == nk - 1))
                    # 1/sums as exp(-ln(sums)) on ACT: both functions live in
                    # the natural_log_exp table set, and this keeps the DVE
                    # free ([1,512] DVE reciprocal is single-lane, ~3.4us)
                    lns = psm.tile([1, 512], fp32, tag="lns")
                    nc.scalar.activation(lns[:], op[HS:HS + 1, :], Ln)
                    inv = psm.tile([1, 512], bf16, tag="inv")
                    nc.scalar.activation(inv[:], lns[:], Exp, scale=-1.0)
                    bc = pmisc.tile([64, 512], fp32, tag="ps")
                    nc.tensor.matmul(bc[:], ones64[:], inv[:], start=True, stop=True)
                    bcs = psm.tile([64, 512], fp32, tag="bcs")
                    nc.vector.tensor_copy(bcs[:], bc[:])
                    nc.vector.tensor_mul(oT[c][pp:pp + 64, 512 * qb:512 * (qb + 1)],
                                         op[0:HS, :], bcs[:])

            # ---- phase 4: output projection ----
            for qt in range(NQT):
                for jh in range(2):
                    ops = pmisc.tile([128, 512], fp32, tag="ps")
                    for vc in range(NCC):
                        nc.tensor.matmul(ops[:],
                                         oT[vc][:, 128 * qt:128 * (qt + 1)],
                                         wo_b[vc][:, 512 * jh:512 * (jh + 1)],
                                         start=(vc == 0), stop=(vc == NCC - 1))
                    osb = pout.tile([128, 512], fp32, tag="outsb")
                    nc.any.tensor_copy(osb[:], ops[:])
                    nc.sync.dma_start(
                        out[128 * qt:128 * (qt + 1), 512 * jh:512 * (jh + 1)],
                        osb[:])

    nc.compile()
    nc.m = get_hw_module(nc.m)
    return nc


def _get_nc():
    if "nc" not in _CACHE:
        _CACHE["nc"] = _build()
    return _CACHE["nc"]


def _make_in_maps(x, W_qkv, W_out):
    x = np.ascontiguousarray(x, dtype=np.float32)
    wqkvT = np.ascontiguousarray(W_qkv.T, dtype=np.float32)
    woT_full = np.ascontiguousarray(W_out.T, dtype=np.float32)
    in_maps = []
    for c in range(N_CORES):
        b, g = c // 2, c % 2
        in_maps.append({
            "xs": np.ascontiguousarray(x[b][:, VSL * g:VSL * (g + 1)]),
            "wqkvT": wqkvT,
            "woT": np.ascontiguousarray(woT_full[VSL * g:VSL * (g + 1), :]),
        })
    return in_maps


def _run(x, W_qkv, W_out, trace=False, tmpdir=None):
    from concourse import bass_utils
    nc = _get_nc()
    in_maps = _make_in_maps(x, W_qkv, W_out)
    kwargs = {}
    if trace:
        bass_utils.upload_artifacts = lambda d: d
        kwargs = dict(trace=True, tmpdir=tmpdir)
    res = bass_utils.run_bass_kernel_spmd(
        nc, in_maps, core_ids=list(range(N_CORES)), **kwargs)
    out = np.empty((B_FULL, S, V_FULL), dtype=np.float32)
    for b in range(B_FULL):
        out[b] = res.results[2 * b]["out"] + res.results[2 * b + 1]["out"]
    return out, res


def kernel(x, W_qkv, W_out):
    out, _ = _run(x, W_qkv, W_out)
    return out


# revision 9
# speedup vs baseline: 1.3529x; 1.3529x over previous
"""Trainium2 Bass kernel for causal multi-head attention (B=4, S=2048, V=1024, H=16).

Sharding over 8 NeuronCores: core c handles batch b = c//2 and head group
g = c%2 (8 heads, i.e. a 512-wide slice of the hidden dim).  Each core
computes QKV projections, causal attention and a partial output projection
against its 512-row slice of W_out.T; the host sums the two partial output
projections per batch element (the tensor-parallel all-reduce).

On-core dataflow (all matmuls in bf16 with fp32 PSUM accumulation):
  x tiles --PE transpose--> xT --matmul--> QT/KT (d-major) and V (s-major)
  scoresT[k,q] = KT_tile.T @ QT  (only causal strips, q >= 128*kt)
  attnT = exp(scores/8)  (ACT, PSUM->SBUF bf16), diagonal 128-block masked
  oT_aug[d|sum, q] = V_aug.T @ attnT  (ones column yields softmax denoms)
  oT = oT_aug * broadcast(1/sum)  (PE rank-1 broadcast + DVE multiply)
  out_partial = oT.T @ W_out.T slice  (PSUM accumulate over v chunks)
"""

import sys

if "/opt/trn_rl_repo" not in sys.path:
    sys.path.insert(0, "/opt/trn_rl_repo")

import numpy as np

S = 2048          # sequence length
HS = 64           # head size
NHL = 8           # heads per core
VSL = 512         # hidden slice per core
NKT = 16          # 128-row key tiles
NCC = 4           # 128-partition chunks of the 512-wide slice
NQT = 16          # 128-row query tiles (output projection)
V_FULL = 1024
B_FULL = 4
N_CORES = 8

_CACHE = {}


def _build():
    import concourse.mybir as mybir
    import concourse.tile as tile
    from concourse import bacc
    from concourse.bass_interp import get_hw_module
    from concourse.masks import make_identity, make_upper_triangular

    fp32 = mybir.dt.float32
    bf16 = mybir.dt.bfloat16
    Exp = mybir.ActivationFunctionType.Exp
    Ln = mybir.ActivationFunctionType.Ln

    nc = bacc.Bacc("TRN2", target_bir_lowering=False, debug=False,
                   num_devices=N_CORES)
    xs = nc.dram_tensor("xs", [S, VSL], fp32, kind="ExternalInput").ap()
    wqkvT = nc.dram_tensor("wqkvT", [HS, 3 * HS], fp32, kind="ExternalInput").ap()
    woT = nc.dram_tensor("woT", [VSL, V_FULL], fp32, kind="ExternalInput").ap()
    out = nc.dram_tensor("out", [S, V_FULL], fp32, kind="ExternalOutput").ap()

    with tile.TileContext(nc) as tc:
        with (
            tc.tile_pool(name="const", bufs=1) as pc,
            tc.tile_pool(name="stage", bufs=3) as pstg,
            tc.tile_pool(name="attn", bufs=2) as pat,
            tc.tile_pool(name="small", bufs=2) as psm,
            tc.tile_pool(name="outp", bufs=2) as pout,
            tc.tile_pool(name="ps_misc", bufs=2, space="PSUM") as pmisc,
            tc.tile_pool(name="ps_scores", bufs=2, space="PSUM") as pscr,
            tc.tile_pool(name="ps_o", bufs=2, space="PSUM") as pso,
        ):
            # ---- constants ----
            ident = pc.tile([128, 128], fp32, tag="ident")
            make_identity(nc, ident[:])
            trimask = pc.tile([128, 128], bf16, tag="trimask")
            make_upper_triangular(nc, trimask[:], val=1.0, diag=True)
            ones64 = pc.tile([1, HS], bf16, tag="ones64")
            nc.gpsimd.memset(ones64[:], 1.0)

            # qkv weights, replicated into both partition halves so lhsT and
            # rhs of the per-head matmuls share a base partition
            wq2f = pc.tile([128, 3 * HS], fp32, tag="wq2f")
            nc.sync.dma_start(wq2f[0:64, :], wqkvT[:, :])
            nc.sync.dma_start(wq2f[64:128, :], wqkvT[:, :])
            wq2 = pc.tile([128, 3 * HS], bf16, tag="wq2")
            nc.vector.tensor_copy(wq2[:], wq2f[:])

            wo_b = []
            for c in range(NCC):
                wof = pstg.tile([128, V_FULL], fp32, tag="wostage")
                nc.sync.dma_start(wof[:], woT[128 * c:128 * (c + 1), :])
                wob = pc.tile([128, V_FULL], bf16, tag=f"wo{c}")
                nc.vector.tensor_copy(wob[:], wof[:])
                wo_b.append(wob)

            # persistent bf16 activations (partition-packed head pairs)
            xT = [pc.tile([128, S], bf16, tag=f"xt{c}", name=f"xt{c}")
                  for c in range(NCC)]
            QT = [pc.tile([128, S], bf16, tag=f"qt{c}", name=f"qt{c}")
                  for c in range(NCC)]
            KT = [pc.tile([128, S], bf16, tag=f"kt{c}", name=f"kt{c}")
                  for c in range(NCC)]
            Vt = [pc.tile([128, NKT, HS + 1], bf16, tag=f"v{h}", name=f"v{h}")
                  for h in range(NHL)]
            oT = [pc.tile([128, S], bf16, tag=f"ot{c}", name=f"ot{c}")
                  for c in range(NCC)]

            # ---- phase 1: transpose x into xT (and downcast to bf16) ----
            for rt in range(NQT):
                xin = pstg.tile([128, VSL], fp32, tag="xin")
                nc.sync.dma_start(xin[:], xs[128 * rt:128 * (rt + 1), :])
                for cc in range(NCC):
                    tp = pmisc.tile([128, 128], fp32, tag="ps")
                    nc.tensor.transpose(tp[:], xin[:, 128 * cc:128 * (cc + 1)],
                                        ident[:])
                    nc.vector.tensor_copy(xT[cc][:, 128 * rt:128 * (rt + 1)], tp[:])

            # ---- phase 2: QKV projections for ALL heads up front ----
            # (writing QT/KT before any score matmuls read them avoids WAR
            # stalls on the shared head-pair chunks, which would bubble the
            # PE once per head and re-throttle HAM)
            for h in range(NHL):
                c, pp = h // 2, 64 * (h % 2)
                xTh = xT[c]
                for j in range(4):
                    qp = pmisc.tile([64, 512], fp32, tag="ps")
                    nc.tensor.matmul(qp[:], wq2[pp:pp + 64, 0:HS],
                                     xTh[pp:pp + 64, 512 * j:512 * (j + 1)],
                                     start=True, stop=True)
                    nc.vector.tensor_copy(QT[c][pp:pp + 64, 512 * j:512 * (j + 1)],
                                          qp[:])
                    kp = pmisc.tile([64, 512], fp32, tag="ps")
                    nc.tensor.matmul(kp[:], wq2[pp:pp + 64, HS:2 * HS],
                                     xTh[pp:pp + 64, 512 * j:512 * (j + 1)],
                                     start=True, stop=True)
                    nc.vector.tensor_copy(KT[c][pp:pp + 64, 512 * j:512 * (j + 1)],
                                          kp[:])
                for t4 in range(NKT // 4):
                    vp = pmisc.tile([128, 4, HS], fp32, tag="ps")
                    for tt in range(4):
                        t = 4 * t4 + tt
                        nc.tensor.matmul(vp[:, tt, :],
                                         xTh[pp:pp + 64, 128 * t:128 * (t + 1)],
                                         wq2[pp:pp + 64, 2 * HS:3 * HS],
                                         start=True, stop=True)
                    nc.vector.tensor_copy(Vt[h][:, 4 * t4:4 * (t4 + 1), 0:HS],
                                          vp[:])
                nc.vector.memset(Vt[h][:, :, HS:HS + 1], 1.0)

            # ---- phase 3: attention, per head ----
            for h in range(NHL):
                c, pp = h // 2, 64 * (h % 2)

                # scoresT strips + exp + causal mask
                strips = []
                for kt in range(NKT):
                    w = S - 128 * kt
                    strip = pat.tile([128, w], bf16, tag=f"st{kt}")
                    strips.append(strip)
                    if kt < 8:
                        pieces = [(0, 1024 - 128 * kt), (1024 - 128 * kt, 1024)]
                    else:
                        pieces = [(0, w)]
                    for off, pw in pieces:
                        sc = pscr.tile([128, 1024], fp32, tag="scr")
                        for ch in range(0, pw, 512):
                            n = min(512, pw - ch)
                            q0 = 128 * kt + off + ch
                            nc.tensor.matmul(
                                sc[:, ch:ch + n],
                                KT[c][pp:pp + 64, 128 * kt:128 * (kt + 1)],
                                QT[c][pp:pp + 64, q0:q0 + n],
                                start=True, stop=True)
                        nc.scalar.activation(strip[:, off:off + pw], sc[:, 0:pw],
                                             Exp, scale=0.125)
                    nc.vector.tensor_mul(strip[:, 0:128], strip[:, 0:128],
                                         trimask[:])

                # attn @ V_aug per 512-wide query block, then normalize
                for qb in range(4):
                    op = pso.tile([HS + 1, 512], fp32, tag="ot")
                    nk = 4 * qb + 4
                    for kt in range(nk):
                        a_off = 512 * qb - 128 * kt
                        o0 = max(0, -a_off)
                        nc.tensor.matmul(
                            op[:, o0:512],
                            Vt[h][:, kt, :],
                            strips[kt][:, max(0, a_off):512 * qb + 512 - 128 * kt],
                            start=(kt == 0), stop=(kt == nk - 1))
                    # 1/sums via the fast Newton reciprocal (51 ULP is ample
                    # for softmax denominators; exact DVE reciprocal on a
                    # single-partition [1,512] costs ~3.4us)
                    sums = psm.tile([1, 512], fp32, tag="sums")
                    nc.vector.tensor_copy(sums[:], op[HS:HS + 1, :])
                    invf = psm.tile([1, 512], fp32, tag="invf")
                    nc.vector.reciprocal_approx_fast(invf[:], sums[:])
                    inv = psm.tile([1, 512], bf16, tag="inv")
                    nc.vector.tensor_copy(inv[:], invf[:])
                    bc = pmisc.tile([64, 512], fp32, tag="ps")
                    nc.tensor.matmul(bc[:], ones64[:], inv[:], start=True, stop=True)
                    bcs = psm.tile([64, 512], fp32, tag="bcs")
                    nc.vector.tensor_copy(bcs[:], bc[:])
                    nc.vector.tensor_mul(oT[c][pp:pp + 64, 512 * qb:512 * (qb + 1)],
                                         op[0:HS, :], bcs[:])

            # ---- phase 4: output projection ----
            for qt in range(NQT):
                for jh in range(2):
                    ops = pmisc.tile([128, 512], fp32, tag="ps")
                    for vc in range(NCC):
                        nc.tensor.matmul(ops[:],
                                         oT[vc][:, 128 * qt:128 * (qt + 1)],
                                         wo_b[vc][:, 512 * jh:512 * (jh + 1)],
                                         start=(vc == 0), stop=(vc == NCC - 1))
                    osb = pout.tile([128, 512], fp32, tag="outsb")
                    nc.any.tensor_copy(osb[:], ops[:])
                    nc.sync.dma_start(
                        out[128 * qt:128 * (qt + 1), 512 * jh:512 * (jh + 1)],
                        osb[:])

    nc.compile()
    nc.m = get_hw_module(nc.m)
    return nc


def _get_nc():
    if "nc" not in _CACHE:
        _CACHE["nc"] = _build()
    return _CACHE["nc"]


def _make_in_maps(x, W_qkv, W_out):
    x = np.ascontiguousarray(x, dtype=np.float32)
    wqkvT = np.ascontiguousarray(W_qkv.T, dtype=np.float32)
    woT_full = np.ascontiguousarray(W_out.T, dtype=np.float32)
    in_maps = []
    for c in range(N_CORES):
        b, g = c // 2, c % 2
        in_maps.append({
            "xs": np.ascontiguousarray(x[b][:, VSL * g:VSL * (g + 1)]),
            "wqkvT": wqkvT,
            "woT": np.ascontiguousarray(woT_full[VSL * g:VSL * (g + 1), :]),
        })
    return in_maps


def _run(x, W_qkv, W_out, trace=False, tmpdir=None):
    from concourse import bass_utils
    nc = _get_nc()
    in_maps = _make_in_maps(x, W_qkv, W_out)
    kwargs = {}
    if trace:
        bass_utils.upload_artifacts = lambda d: d
        kwargs = dict(trace=True, tmpdir=tmpdir)
    res = bass_utils.run_bass_kernel_spmd(
        nc, in_maps, core_ids=list(range(N_CORES)), **kwargs)
    out = np.empty((B_FULL, S, V_FULL), dtype=np.float32)
    for b in range(B_FULL):
        out[b] = res.results[2 * b]["out"] + res.results[2 * b + 1]["out"]
    return out, res


def kernel(x, W_qkv, W_out):
    out, _ = _run(x, W_qkv, W_out)
    return out
